# revision 51
# baseline (speedup 1.0000x reference)
"""GQA causal attention (B=1, S=4096, D=1024, H=16, HKV=4, Dh=64, RoPE) on
8 Trainium2 NeuronCores.

Sharding: 8-way head parallelism. Core c owns query heads {2c, 2c+1} (which
share one KV head, g = c//2) and all 4096 query positions, so every core runs
the SAME program (one NEFF shared by all 8 cores) and only the weight shards
passed as inputs differ. Each core produces a partial output projection
[4096, 1024] (fp16, its heads' slice of wo); the host sums the 8 partials in
float64.

Device program (v2 — software-pipelined, fp16/bf16 datapath):
  All HBM traffic is fp16 and batched into few large DMAs (the DMA engines and
  the HWDGE descriptor generator are serial shared resources): x^T arrives as
  8 [128, 4096]-fp16 loads (one per 512-column group), tables/weights as one
  load each, the output as one [128, 1024]-fp16 store per 128-row block.

  Phase A (projections) is interleaved INTO phase B (attention) as "filler"
  units: the prologue projects column groups 0-1 (enough for q-tile 0), then
  groups {2,3}, {4,5}, {6,7} are loaded + projected + roped during q-tiles 0,
  1, 2 respectively, hiding all projection DMA/compute behind attention.
  Q and KV projections are merged where possible: K (64 rows) and V (64 rows)
  share one [128, 512] matmul chain per group. RoPE (rotate-half mapped to
  adjacent-pair shuffle via a host-side permutation of the weight rows) is
  applied by DVE reading the projection PSUM directly.

  Phase B per q-tile of 1024 columns, per head: per 128-key chunk,
  S^T = K^T_chunk^T @ Q^T (causal suffix only, fp16 in / fp32 PSUM out), exp
  on ScalarE reading PSUM (softmax is shift-invariant and scores are bounded
  << 88, so no row-max pass; fixed bias -10) written as bf16 (range!), the
  within-chunk upper triangle zeroed by gpsimd affine_select, then
  P@V accumulated in PSUM with a ones-column appended to V so row 64 collects
  the softmax denominator. Normalization: DVE copies the accumulator out,
  reciprocal on DVE, the per-column reciprocal row is broadcast across
  partitions by gpsimd (Pool engine — otherwise idle), DVE multiplies into
  the normalized ON buffer (fp16). The output projection of tile t
  (ON^T slices @ wo -> fp16 partials) is spread across tile t+1's chunks.

  Scheduling notes (learned against the TimelineSim cost model + real HW):
  PE is the bottleneck engine (~163us busy: scores+PV 113us, projections
  27us, out-proj 14us + overheads) with ScalarE exp second (~148us), so
  phase-A/oproj fillers are woven between chunks; each chunk's PV matmul is
  deferred one chunk (two at head switches) so PE never stalls on exp; junk
  transposes warm the PE p-state ramp during the DMA-bound prologue; the
  tail normalization is pipelined per 512-column segment. Hardware-found
  constraints: GPSIMD cannot touch PSUM, gpsimd partition_broadcast sources
  the TILE's partition 0 (not the AP's), reciprocal_approx_fast must read
  SBUF, and dma_start_transpose corrupts strided sub-tile destinations (PE
  transposes are used for V instead). Measured: 218us vs 282us for v1,
  rel err 4e-3 (fp16/bf16 rounding) vs 1.6e-3.

If the mask input is NOT the standard causal mask, the v1 dense fallback
program (all chunks, explicit mask add before exp, fp32 datapath) is compiled
instead: slower, still correct for any additive mask.
"""

import os

import numpy as np

B, S, D = 1, 4096, 1024
H, HKV, DH = 16, 4, 64
HPC = 2             # query heads per core
NCORES = 8
ROPE_THETA = 10000.0
QT_TILE = 1024      # q columns per attention tile
EXP_BIAS = -10.0    # shift inside exp; softmax-invariant, adds overflow headroom

NSG = S // 512      # 8 column groups
NCH_D = D // 128    # 8 contraction chunks for projections
NKCH = S // 128     # 32 key chunks
NQT = S // QT_TILE  # 4 q-tiles

_cache = {}


def _build_fast():
    """Causal-mask program (the fast path)."""
    import concourse.bass as bass
    import concourse.tile as tile
    from concourse import bacc, mybir
    from concourse.masks import make_identity

    f32 = mybir.dt.float32
    f16 = mybir.dt.float16
    bf16 = mybir.dt.bfloat16

    nc = bacc.Bacc(None, target_bir_lowering=False)

    # ---- DRAM I/O (all fp16) ----
    xS = nc.dram_tensor("xS", [128, NSG * NCH_D * 512], f16, kind="ExternalInput")
    wqS = nc.dram_tensor("wqS", [128, NCH_D * 128], f16, kind="ExternalInput")
    wkvS = nc.dram_tensor("wkvS", [128, NCH_D * 128], f16, kind="ExternalInput")
    woT = nc.dram_tensor("woT", [128, D], f16, kind="ExternalInput")
    cosT = nc.dram_tensor("cosT", [128, S], f16, kind="ExternalInput")
    sinTs = nc.dram_tensor("sinTs", [128, S], f16, kind="ExternalInput")
    out = nc.dram_tensor("out", [S, D], f16, kind="ExternalOutput")
    dbg = bool(os.environ.get("KDBG"))
    if dbg:
        dbg_qtr = nc.dram_tensor("dbg_qtr", [128, S], f16, kind="ExternalOutput")
        dbg_ktr = nc.dram_tensor("dbg_ktr", [128, S], f16, kind="ExternalOutput")
        dbg_on = nc.dram_tensor("dbg_on", [128, S], f16, kind="ExternalOutput")
        dbg_vp = nc.dram_tensor("dbg_vp", [128, NKCH * (DH + 2)], bf16,
                                kind="ExternalOutput")
        dbg_den = nc.dram_tensor("dbg_den", [NQT * HPC, QT_TILE], f32,
                                 kind="ExternalOutput")

    with tile.TileContext(nc) as tc:
        with tc.tile_pool(name="const", bufs=1) as cpool, \
             tc.tile_pool(name="xs", bufs=2) as xs_pool, \
             tc.tile_pool(name="rtmp", bufs=2) as rtmp, \
             tc.tile_pool(name="vtt", bufs=2) as vtt_pool, \
             tc.tile_pool(name="esb", bufs=1) as e_pool, \
             tc.tile_pool(name="osb", bufs=2) as ot_pool, \
             tc.tile_pool(name="obst", bufs=4) as ob_pool, \
             tc.tile_pool(name="sps", bufs=1, space="PSUM") as s_ps_pool, \
             tc.tile_pool(name="ops", bufs=1, space="PSUM") as o_ps_pool, \
             tc.tile_pool(name="msc", bufs=1, space="PSUM") as misc_pool:

            # ---- resident constants / accumulators ----
            wq_sb = cpool.tile([128, NCH_D * 128], f16)
            wkv_sb = cpool.tile([128, NCH_D * 128], f16)
            wo_sb = cpool.tile([128, D], f16)
            cos_sb = cpool.tile([128, S], f16)
            sin_sb = cpool.tile([128, S], f16)
            QTr = cpool.tile([128, S], f16)        # rope(Q)^T, rows 0-63 h0, 64-127 h1
            KTr = cpool.tile([128, S], f16)        # rope(K)^T, duplicated in both halves
            Vp = cpool.tile([128, NKCH, DH + 2], bf16)  # V chunks + ones column
            ON = cpool.tile([128, S], f16)         # normalized O^T
            ident = cpool.tile([DH, DH], f32)
            biasc = cpool.tile([128, 1], f32)
            ones_row = cpool.tile([128, DH], f32)

            make_identity(nc, ident[:, :])
            nc.vector.memset(biasc, float(EXP_BIAS))
            nc.vector.memset(ones_row, 1.0)
            nc.vector.memset(Vp[:, :, DH:DH + 1], 1.0)

            # ---- constant DMAs (scalar queue; ordered for earliest phase B
            #      start: xs0, the first table halves, then the rest) ----
            HS = 1024
            nc.scalar.dma_start(out=wq_sb, in_=wqS[:, :])

            def dma_xs(g):
                xs = xs_pool.tile([128, NCH_D * 512], f16, tag="xs",
                                  name=f"xs_{g}")
                nc.sync.dma_start(out=xs, in_=xS[:, g * NCH_D * 512:
                                                (g + 1) * NCH_D * 512])
                return xs

            xs_tiles = {}
            xs_tiles[0] = dma_xs(0)
            nc.scalar.dma_start(out=wkv_sb, in_=wkvS[:, :])
            nc.scalar.dma_start(out=cos_sb[:, 0:HS], in_=cosT[:, 0:HS])
            nc.scalar.dma_start(out=sin_sb[:, 0:HS], in_=sinTs[:, 0:HS])
            xs_tiles[1] = dma_xs(1)
            nc.scalar.dma_start(out=cos_sb[:, HS:S], in_=cosT[:, HS:S])
            nc.scalar.dma_start(out=sin_sb[:, HS:S], in_=sinTs[:, HS:S])
            nc.scalar.dma_start(out=wo_sb, in_=woT[:, :])

            # misc PSUM: two 1-bank slots, round-robin for all filler matmuls
            misc_state = [0]

            def misc_tile(name):
                tag = "mt"[misc_state[0]]
                misc_state[0] ^= 1
                return misc_pool.tile([128, 512], f32, tag=tag, name=name)

            # PE p-state warmup: the tensor engine only reaches full clock
            # after 3us of continuous execution; burn junk transposes during
            # the (DMA-bound) prologue so the first projections run at speed.
            for w in range(3):
                wt = misc_tile(f"warm{w}")
                for j in range(8):
                    nc.tensor.transpose(wt[0:64, j * 64:(j + 1) * 64],
                                        ones_row[0:64, :],
                                        ident[:, :])

            # ---- phase A unit generators (per 512-column group) ----
            SHUF = [i ^ 1 for i in range(32)]

            def rope_from_psum(ps_ap, sb_out_ap, scols, nrow, dup_out=None):
                m1 = rtmp.tile([128, 512], f16, tag="rope_m1")
                m2 = rtmp.tile([128, 512], f16, tag="rope_m2")
                sh = rtmp.tile([128, 512], f16, tag="rope_sh")
                nc.vector.tensor_mul(m1[0:nrow, :], ps_ap, cos_sb[0:nrow, scols])
                nc.vector.tensor_mul(m2[0:nrow, :], ps_ap, sin_sb[0:nrow, scols])
                nc.vector.stream_shuffle(sh[0:nrow, :], m2[0:nrow, :], SHUF)
                nc.vector.tensor_add(sb_out_ap, m1[0:nrow, :], sh[0:nrow, :])
                if dup_out is not None:
                    nc.vector.tensor_add(dup_out, m1[0:nrow, :], sh[0:nrow, :])

            def group_units(g):
                """Return list of emission closures for phase-A group g
                (xs DMA must already have been issued; xs_tiles[g] set)."""
                scols = bass.ds(g * 512, 512)
                st = {}

                def u_qmm():
                    st["qt"] = misc_tile(f"qt_{g}")
                    xs = xs_tiles[g]
                    for cd in range(NCH_D):
                        nc.tensor.matmul(st["qt"][:, :],
                                         wq_sb[:, cd * 128:(cd + 1) * 128],
                                         xs[:, cd * 512:(cd + 1) * 512],
                                         start=(cd == 0), stop=(cd == NCH_D - 1))

                def u_qrope():
                    rope_from_psum(st["qt"][:, :], QTr[:, scols], scols, 128)

                def u_kvmm():
                    st["kv"] = misc_tile(f"kv_{g}")
                    xs = xs_tiles[g]
                    for cd in range(NCH_D):
                        nc.tensor.matmul(st["kv"][:, :],
                                         wkv_sb[:, cd * 128:(cd + 1) * 128],
                                         xs[:, cd * 512:(cd + 1) * 512],
                                         start=(cd == 0), stop=(cd == NCH_D - 1))

                def u_krope():
                    rope_from_psum(st["kv"][0:64, :], KTr[0:64, scols], scols,
                                   64, dup_out=KTr[64:128, scols])

                def u_vcopy():
                    vt = vtt_pool.tile([64, 512], f32, tag="vt")
                    st["vt"] = vt
                    nc.vector.tensor_copy(vt, st["kv"][64:128, :])

                def u_vtrans():
                    tr = misc_tile(f"tr_{g}")
                    for j in range(4):
                        nc.tensor.transpose(tr[:, j * 64:(j + 1) * 64],
                                            st["vt"][:, j * 128:(j + 1) * 128],
                                            ident[:, :])
                    nc.vector.tensor_copy(
                        Vp[:, g * 4:(g + 1) * 4, 0:DH],
                        tr[:, 0:4 * DH].rearrange("p (j d) -> p j d", j=4))

                return [u_qmm, u_qrope, u_kvmm, u_krope, u_vcopy, u_vtrans]

            # ---- phase B helpers ----
            # (GPSIMD cannot read PSUM, so staging copies go DVE / ScalarE)
            COPY_ENGINES = [
                lambda o, i: nc.vector.tensor_copy(o, i),
                lambda o, i: nc.scalar.copy(o, i),
            ]

            def emit_oproj(t, qsub, dseg, ob_state, psum_tile=None,
                           eng_idx=None):
                """One output-projection unit: [128 q, 512 d] partial."""
                qg = t * (QT_TILE // 128) + qsub
                if dseg == 0:
                    ob_state[qg] = ob_pool.tile([128, D], f16, tag="ob",
                                                name=f"ob_{qg}")
                op = psum_tile if psum_tile is not None \
                    else misc_tile(f"op_{qg}_{dseg}")
                nc.tensor.matmul(
                    op[:, 0:512],
                    ON[:, qg * 128:(qg + 1) * 128],
                    wo_sb[:, dseg * 512:(dseg + 1) * 512],
                    start=True, stop=True)
                if eng_idx is None:
                    eng_idx = 1 if (os.environ.get("KACTCP")
                                    and t < 2 and (qg + dseg) % 2 == 0) else 0
                cp = COPY_ENGINES[eng_idx]
                cp(ob_state[qg][:, dseg * 512:(dseg + 1) * 512], op[:, 0:512])
                if dseg == (D // 512) - 1:
                    nc.sync.dma_start(
                        out=out[qg * 128:(qg + 1) * 128, :],
                        in_=ob_state[qg])

            def emit_norm(t, h, o_ps, seg_hook=None, direct=False):
                """Normalize head h of tile t: ON[64h:64h+64, tile cols] =
                o_ps numerators * (1/denominator row). direct=True (last
                head only) skips the PSUM->SBUF staging copy — o_ps is not
                needed for a next head, so DVE reads it in place."""
                q0 = t * QT_TILE
                rc = ot_pool.tile([DH + 1, QT_TILE], f32, tag="rc",
                                  name=f"rc_{t}_{h}")
                ot = ot_pool.tile([DH + 1, QT_TILE], f32, tag="ot",
                                  name=f"ot_{t}_{h}")
                rr = ot_pool.tile([1, QT_TILE], f32, tag="rr",
                                  name=f"rr_{t}_{h}")
                if direct:
                    # tail head: per-segment pipeline so bcast/mul/oproj of
                    # segment 0 overlap the copy/recip of segment 1
                    for seg in range(QT_TILE // 512):
                        cs = bass.ds(seg * 512, 512)
                        nc.vector.tensor_copy(ot[:, cs], o_ps[:, cs])
                        nc.vector.reciprocal_approx_fast(rc[:, cs],
                                                         ot[:, cs])
                        nc.vector.tensor_copy(rr[0:1, cs], rc[DH:DH + 1, cs])
                        bc = ot_pool.tile([DH, 512], f32, tag=f"bc{seg}",
                                          name=f"bcT_{t}_{h}_{seg}")
                        nc.gpsimd.partition_broadcast(bc[:, :], rr[0:1, cs],
                                                      channels=DH)
                        nc.vector.tensor_mul(
                            ON[64 * h:64 * h + 64,
                               q0 + seg * 512:q0 + (seg + 1) * 512],
                            ot[0:DH, seg * 512:(seg + 1) * 512], bc[:, :])
                        if seg_hook is not None:
                            seg_hook(seg)
                    return
                nc.vector.tensor_copy(ot, o_ps[:, :])
                nc.vector.reciprocal_approx_fast(rc, ot[:, :])
                # partition_broadcast sources the TILE's partition 0, so
                # the 1/denominator row must be staged to its own tile
                (nc.gpsimd if os.environ.get('KRRP') else nc.vector).tensor_copy(rr[0:1, :], rc[DH:DH + 1, :])
                if dbg:
                    nc.sync.dma_start(out=dbg_den[t * HPC + h:t * HPC + h + 1, :],
                                      in_=ot[DH:DH + 1, :])
                for seg in range(QT_TILE // 512):
                    cs = bass.ds(seg * 512, 512)
                    bc = ot_pool.tile([DH, 512], f32, tag=f"bc{seg}",
                                      name=f"bc_{t}_{h}_{seg}")
                    nc.gpsimd.partition_broadcast(bc[:, :], rr[0:1, cs],
                                                  channels=DH)
                    nc.vector.tensor_mul(
                        ON[64 * h:64 * h + 64,
                           q0 + seg * 512:q0 + (seg + 1) * 512],
                        ot[0:DH, seg * 512:(seg + 1) * 512], bc[:, :])
                    if seg_hook is not None:
                        seg_hook(seg)

            # Global deferred-PV stream state: PE order per chunk is
            # [score(c), PV(c-1)], so PE never waits on exp(c) — it always has
            # the previous chunk's PV (whose exp finished during score(c)).
            # A head's FIRST PV is deferred one extra chunk: it write-after-
            # read depends on the previous head's accumulator staging copy
            # (DVE), which needs the extra slack.
            pv_q = []      # [t, h, c, qs, e_sb, first, lastc, age]
            pv_o = [None]  # current PSUM accumulator

            def pump_pv(force=False, norm_seg_hook=None):
                while pv_q:
                    t, h, c, qs, e_sb, first, lastc, age = pv_q[0]
                    if not (force or age >= 2 or (age >= 1 and not first)):
                        return
                    pv_q.pop(0)
                    if first:
                        pv_o[0] = o_ps_pool.tile(
                            [DH + 1, QT_TILE], f32, tag="oacc",
                            name=f"ops_{t}_{h}")
                    for lo, hi in ((qs, 512), (max(qs, 512), QT_TILE)):
                        if lo >= hi:
                            continue
                        cs = bass.ds(lo, hi - lo)
                        nc.tensor.matmul(
                            pv_o[0][:, cs], Vp[:, c, 0:DH + 1],
                            e_sb[:, cs],
                            start=first, stop=lastc)
                    if lastc:
                        last = (t == NQT - 1 and h == HPC - 1)
                        emit_norm(t, h, pv_o[0], seg_hook=norm_seg_hook,
                                  direct=last)

            def chunk_units(t, h, nch):
                """Emission closures for all key-chunks of (tile t, head h).
                Chunks are emitted with the short diagonal chunks interleaved
                among the full-height ones (PV accumulation is commutative),
                so the short chunks' dependency latency hides under the long
                chunks' exp time."""
                q0 = t * QT_TILE
                full = list(range(0, 8 * t))
                diag = list(range(8 * t, nch))
                order = []
                if full and os.environ.get("KILV"):
                    r = max(1, len(full) // len(diag))
                    di = 0
                    for k, c in enumerate(full):
                        order.append(c)
                        if k % r == r - 1 and di < len(diag):
                            order.append(diag[di])
                            di += 1
                    order += diag[di:]
                else:
                    order = full + diag

                def mk(ci, c, first, lastc):
                    def u():
                        qs = max(0, c * 128 - q0)
                        s_ps = s_ps_pool.tile([128, QT_TILE], f32,
                                              tag=f"s{ci % 2}",
                                              name=f"s_{t}_{h}_{ci}")
                        for lo, hi in ((qs, 512), (max(qs, 512), QT_TILE)):
                            if lo >= hi:
                                continue
                            nc.tensor.matmul(
                                s_ps[:, bass.ds(lo, hi - lo)],
                                KTr[64 * h:64 * h + 64, c * 128:(c + 1) * 128],
                                QTr[64 * h:64 * h + 64, q0 + lo:q0 + hi],
                                start=True, stop=True)
                        for p in pv_q:
                            p[7] += 1
                        pump_pv()
                        e_sb = e_pool.tile([128, QT_TILE], bf16,
                                           tag=f"e{ci % 3}",
                                           name=f"e_{t}_{h}_{ci}")
                        nc.scalar.activation(
                            e_sb[:, qs:QT_TILE], s_ps[:, qs:QT_TILE],
                            mybir.ActivationFunctionType.Exp,
                            bias=biasc[:, :], scale=1.0)
                        if c * 128 >= q0:
                            nc.gpsimd.affine_select(
                                out=e_sb[:, qs:qs + 128],
                                in_=e_sb[:, qs:qs + 128],
                                pattern=[[1, 128]],
                                compare_op=mybir.AluOpType.is_ge,
                                fill=0.0, base=0, channel_multiplier=-1)
                        pv_q.append([t, h, c, qs, e_sb, first, lastc, 0])
                    return u

                return [mk(ci, c, ci == 0, ci == nch - 1)
                        for ci, c in enumerate(order)]

            # ---- prologue: just enough for tile-0 chunk 0 — group 0 fully,
            #      group 1's Q projection + rope. Group 1's K/V (first needed
            #      by chunk 4) moves into tile 0's first filler slots. ----
            g0u = group_units(0)
            g1u = group_units(1)
            for u in (g0u[0], g0u[1],            # qt0, ropeQ0
                      g0u[2], g0u[3], g0u[4],    # kv0, ropeK0, vcopy0
                      g1u[0], g1u[1],            # qt1, ropeQ1
                      g0u[5]):                   # tr0 (dma)
                u()
            pre_fillers = [g1u[2], g1u[3], g1u[4], g1u[5]]

            # ---- main loop: tiles with woven fillers ----
            # group g is loaded+projected during tile (g-2)//2 wait... groups
            # 2..7 are spread {t0: g2,g3-dma, t1: g3,g4,g5-dma, ...} — see
            # TILE_GROUPS; dma for group g is issued right after group (g-2)'s
            # last unit so its xs-pool slot is free and the load hides.
            TILE_GROUPS = {0: [2, 3], 1: [4, 5], 2: [6, 7], 3: []}
            ob_state = {}
            pending_op = []   # oproj args from previous tile

            xs_tiles[2] = dma_xs(2)
            xs_tiles[3] = dma_xs(3)
            for t in range(NQT):
                nch = (t + 1) * (QT_TILE // 128)

                fillers = []
                if t == 0:
                    fillers += pre_fillers
                ops = [lambda tp=tp, q=qsub, d=dseg:
                       emit_oproj(tp, q, d, ob_state)
                       for (tp, qsub, dseg) in pending_op]
                pending_op = []
                # a couple of (dependency-free) oproj units first, then the
                # phase-A group units (their xs arrived a tile ago), with the
                # next groups' xs loads issued as their slots free up
                fillers += ops[:2]
                ops = ops[2:]
                for g in TILE_GROUPS[t]:
                    fillers += group_units(g)
                    if 4 <= g + 2 <= 7:
                        fillers.append(lambda g2=g + 2: xs_tiles.__setitem__(
                            g2, dma_xs(g2)))
                    nops = 6 if g % 2 == 0 else len(ops)
                    fillers += ops[:nops]
                    ops = ops[nops:]
                fillers += ops

                chunks = chunk_units(t, 0, nch) + chunk_units(t, 1, nch)
                n = len(chunks)
                m = len(fillers)
                lead = 1
                span = max(1, n - lead - 2)
                # keep fillers out of the PE queue around head switches so the
                # next head's first score matmul issues immediately (ACT would
                # otherwise stall behind a filler projection)
                nofill = (set() if os.environ.get('KNONF') else {nch - 2, nch - 1, nch, 2 * nch - 2, 2 * nch - 1})
                fi = 0
                for i, ce in enumerate(chunks):
                    ce()
                    if i in nofill:
                        continue
                    tgt = 0 if i < lead else min(
                        m, (m * (i - lead + 1) + span - 1) // span)
                    while fi < tgt:
                        fillers[fi]()
                        fi += 1
                while fi < m:
                    fillers[fi]()
                    fi += 1

                pending_op = [(t, qsub, dseg)
                              for qsub in range(QT_TILE // 128)
                              for dseg in range(D // 512)]

            # ---- tail: flush last PV + norm, then the last tile's output
            #      projection with deep PSUM rotation (s banks are free now)
            #      and copies spread across DVE/Pool/ScalarE (all idle) ----
            tail_ops = pending_op
            tail_i = [0]

            def tail_psum(name):
                i = tail_i[0]
                if i % 4 < 2:
                    return misc_tile(name)
                return s_ps_pool.tile([128, QT_TILE], f32,
                                      tag=f"s{i % 2}", name=name)

            def tail_emit(seg):
                # oproj units whose q-block lies in this 512-col segment
                for (tp, qsub, dseg) in tail_ops:
                    if qsub // 4 != seg:
                        continue
                    emit_oproj(tp, qsub, dseg, ob_state,
                               psum_tile=tail_psum(f"top_{qsub}_{dseg}"),
                               eng_idx=tail_i[0] % 2)
                    tail_i[0] += 1

            pump_pv(force=True, norm_seg_hook=tail_emit)

            if dbg:
                nc.sync.dma_start(out=dbg_qtr[:, :], in_=QTr[:, :])
                nc.sync.dma_start(out=dbg_ktr[:, :], in_=KTr[:, :])
                nc.sync.dma_start(out=dbg_on[:, :], in_=ON[:, :])
                nc.sync.dma_start(
                    out=dbg_vp[:, :],
                    in_=Vp[:, :, :].rearrange("p a b -> p (a b)"))

    nc.compile()
    return nc


def _build_dense():
    """Fallback for a non-causal additive mask (v1 program, fp32 datapath)."""
    import concourse.bass as bass
    import concourse.tile as tile
    from concourse import bacc, mybir
    from concourse.masks import make_identity

    f32 = mybir.dt.float32
    f16 = mybir.dt.float16
    f32r = mybir.dt.float32r

    nc = bacc.Bacc(None, target_bir_lowering=False)

    xT = nc.dram_tensor("xT", [D, S], f32r, kind="ExternalInput")
    wqT = nc.dram_tensor("wqT", [D, 128], f32r, kind="ExternalInput")
    wkTd = nc.dram_tensor("wkTd", [D, 128], f32r, kind="ExternalInput")
    wvT = nc.dram_tensor("wvT", [D, DH], f32r, kind="ExternalInput")
    woT = nc.dram_tensor("woT", [128, D], f32r, kind="ExternalInput")
    cosT = nc.dram_tensor("cosT", [128, S], f32, kind="ExternalInput")
    sinTs = nc.dram_tensor("sinTs", [128, S], f32, kind="ExternalInput")
    maskT = nc.dram_tensor("maskT", [S, S], f32, kind="ExternalInput")
    out = nc.dram_tensor("out", [S, D], f16, kind="ExternalOutput")

    from contextlib import ExitStack
    with tile.TileContext(nc) as tc, ExitStack() as phase_a:
        with tc.tile_pool(name="const", bufs=1) as cpool, \
             tc.tile_pool(name="xs", bufs=4) as xs_pool, \
             tc.tile_pool(name="rtmp", bufs=2) as rtmp, \
             tc.tile_pool(name="vtt", bufs=2) as vtt_pool, \
             tc.tile_pool(name="esb", bufs=2) as e_pool, \
             tc.tile_pool(name="osb", bufs=2) as ot_pool, \
             tc.tile_pool(name="mtile", bufs=2) as m_pool:
            prj_ps = phase_a.enter_context(tc.tile_pool(name="prj", bufs=2, space="PSUM"))
            trp_ps = phase_a.enter_context(tc.tile_pool(name="trp", bufs=2, space="PSUM"))

            wq_sb = cpool.tile([128, NCH_D, 128], f32r)
            wk_sb = cpool.tile([128, NCH_D, 128], f32r)
            wv_sb = cpool.tile([128, NCH_D, DH], f32r)
            wo_sb = cpool.tile([128, D], f32r)
            cos_sb = cpool.tile([128, S], f32)
            sin_sb = cpool.tile([128, S], f32)
            QTr = cpool.tile([128, S], f32r)
            KTr = cpool.tile([128, S], f32r)
            Vp = cpool.tile([128, NKCH, DH + 1], f32r)
            ON = cpool.tile([128, S], f32r)
            ident = cpool.tile([DH, DH], f32)
            ones_row = cpool.tile([128, DH], f32)
            biasc = cpool.tile([128, 1], f32)

            for cd in range(NCH_D):
                nc.scalar.dma_start(out=wq_sb[:, cd, :], in_=wqT[cd * 128:(cd + 1) * 128, :])
                nc.scalar.dma_start(out=wk_sb[:, cd, :], in_=wkTd[cd * 128:(cd + 1) * 128, :])
                nc.scalar.dma_start(out=wv_sb[:, cd, :], in_=wvT[cd * 128:(cd + 1) * 128, :])
            for sg in range(NSG):
                sl = bass.ds(sg * 512, 512)
                nc.scalar.dma_start(out=cos_sb[:, sl], in_=cosT[:, sg * 512:(sg + 1) * 512])
                nc.scalar.dma_start(out=sin_sb[:, sl], in_=sinTs[:, sg * 512:(sg + 1) * 512])
            nc.scalar.dma_start(out=wo_sb, in_=woT[:, :])
            make_identity(nc, ident[:, :])
            nc.vector.memset(ones_row, 1.0)
            nc.vector.memset(biasc, float(EXP_BIAS))
            nc.vector.memset(Vp[:, :, DH:DH + 1].bitcast(f32), 1.0)

            def rope_from_psum(ps_ap, sb_out_ap, scols, width):
                m1 = rtmp.tile([128, 512], f32, tag="rope_m1")
                m2 = rtmp.tile([128, 512], f32, tag="rope_m2")
                sh = rtmp.tile([128, 512], f32, tag="rope_sh")
                nc.vector.tensor_mul(m1[:, :width], ps_ap, cos_sb[:, scols])
                nc.vector.tensor_mul(m2[:, :width], ps_ap, sin_sb[:, scols])
                nc.vector.stream_shuffle(sh[:, :width], m2[:, :width],
                                         [i ^ 1 for i in range(32)])
                nc.vector.tensor_add(sb_out_ap, m1[:, :width], sh[:, :width])

            for sg in range(NSG):
                scols = bass.ds(sg * 512, 512)
                qt_ps = prj_ps.tile([128, 512], f32, tag="qt")
                kt_ps = prj_ps.tile([128, 512], f32, tag="kt")
                vt_ps = prj_ps.tile([DH, 512], f32, tag="vt")
                for cd in range(NCH_D):
                    xsl = xs_pool.tile([128, 512], f32r)
                    nc.sync.dma_start(out=xsl, in_=xT[cd * 128:(cd + 1) * 128,
                                                      sg * 512:(sg + 1) * 512])
                    st = (cd == 0)
                    sp = (cd == NCH_D - 1)
                    nc.tensor.matmul(qt_ps[:, :], wq_sb[:, cd, :], xsl,
                                     start=st, stop=sp)
                    nc.tensor.matmul(kt_ps[:, :], wk_sb[:, cd, :], xsl,
                                     start=st, stop=sp)
                    nc.tensor.matmul(vt_ps[:, :], wv_sb[:, cd, :], xsl,
                                     start=st, stop=sp)
                rope_from_psum(qt_ps[:, :], QTr[:, scols], scols, 512)
                rope_from_psum(kt_ps[:, :], KTr[:, scols], scols, 512)
                vt_sb = vtt_pool.tile([DH, 512], f32)
                nc.scalar.copy(vt_sb, vt_ps[:, :])
                for j in range(4):
                    kc = sg * 4 + j
                    tr = trp_ps.tile([128, DH], f32)
                    nc.tensor.transpose(tr[:, :], vt_sb[:, j * 128:(j + 1) * 128],
                                        ident[:, :])
                    nc.vector.tensor_copy(Vp[:, kc, 0:DH], tr[:, :])

            phase_a.close()
            phase_b = ExitStack()
            s_ps_pool = phase_b.enter_context(tc.tile_pool(name="sps", bufs=1, space="PSUM"))
            o_ps_pool = phase_b.enter_context(tc.tile_pool(name="ops", bufs=1, space="PSUM"))

            def emit_oproj(qsub, dseg):
                op = o_ps_pool.tile([128, 512], f32, tag=f"op{dseg}",
                                    name=f"op_{qsub}_{dseg}")
                nc.tensor.matmul(
                    op[:, :],
                    ON[:, qsub * 128:(qsub + 1) * 128],
                    wo_sb[:, dseg * 512:(dseg + 1) * 512],
                    start=True, stop=True)
                ob = m_pool.tile([128, 512], f16, tag="ostage")
                nc.vector.tensor_copy(ob, op[:, :])
                nc.sync.dma_start(
                    out=out[qsub * 128:(qsub + 1) * 128,
                            dseg * 512:(dseg + 1) * 512],
                    in_=ob)

            pending = []

            def emit_norm(tn, hn, o_psn):
                ot = ot_pool.tile([DH + 1, QT_TILE], f32, tag="ot",
                                  name=f"ot_{tn}_{hn}")
                nc.vector.tensor_copy(ot, o_psn[:, :])
                q0n = tn * QT_TILE
                rc = ot_pool.tile([DH + 1, QT_TILE], f32, tag="rc",
                                  name=f"rc_{tn}_{hn}")
                nc.vector.reciprocal_approx_fast(rc, ot[:, :])
                for seg in range(QT_TILE // 512):
                    cs = bass.ds(seg * 512, 512)
                    rbseg = o_ps_pool.tile([128, 512], f32, tag=f"op{seg}",
                                           name=f"rb_{tn}_{hn}_{seg}")
                    nc.tensor.matmul(rbseg[0:DH, :],
                                     ones_row[DH:DH + 1, :],
                                     rc[DH:DH + 1, cs],
                                     start=True, stop=True)
                    nc.vector.tensor_mul(
                        ON[64 * hn:64 * hn + 64,
                           q0n + seg * 512:q0n + (seg + 1) * 512],
                        ot[0:DH, seg * 512:(seg + 1) * 512], rbseg[0:DH, :])

            for t in range(NQT):
                q0 = t * QT_TILE
                nch = NKCH
                for h in range(HPC):
                    o_ps = o_ps_pool.tile([DH + 1, QT_TILE], f32, tag="oacc",
                                          name=f"ops_{t}_{h}")
                    for ci in range(nch):
                        c = ci
                        s_ps = s_ps_pool.tile([128, QT_TILE], f32,
                                              tag=f"s{ci % 2}",
                                              name=f"s_{t}_{h}_{ci}")
                        lhs = KTr[64 * h:64 * h + 64, c * 128:(c + 1) * 128]
                        for lo, hi in ((0, 512), (512, QT_TILE)):
                            nc.tensor.matmul(
                                s_ps[:, bass.ds(lo, hi - lo)], lhs,
                                QTr[64 * h:64 * h + 64, q0 + lo:q0 + hi],
                                start=True, stop=True)
                        e_sb = e_pool.tile([128, QT_TILE], f32r, tag=f"e{h}",
                                           name=f"e_{t}_{h}_{ci}")
                        sm = m_pool.tile([128, QT_TILE], f32, tag="mask")
                        nc.sync.dma_start(
                            out=sm, in_=maskT[c * 128:(c + 1) * 128,
                                              q0:q0 + QT_TILE])
                        sms = m_pool.tile([128, QT_TILE], f32, tag="masked")
                        nc.vector.tensor_add(sms, s_ps[:, :], sm)
                        nc.scalar.activation(
                            e_sb[:, :], sms,
                            mybir.ActivationFunctionType.Exp,
                            bias=biasc[:, :], scale=1.0)
                        for lo, hi in ((0, 512), (512, QT_TILE)):
                            cs = bass.ds(lo, hi - lo)
                            nc.tensor.matmul(
                                o_ps[:, cs], Vp[:, c, :],
                                e_sb[:, cs],
                                start=(c == 0), stop=(c == nch - 1))
                        if pending:
                            emit_oproj(*pending.pop(0))
                    emit_norm(t, h, o_ps)
                for j in range(QT_TILE // 128):
                    for dseg in range(D // 512):
                        pending.append((t * (QT_TILE // 128) + j, dseg))
            while pending:
                emit_oproj(*pending.pop(0))
            phase_b.close()

    nc.compile()
    return nc


def _rope_tables():
    """Pair-interleaved rope tables + the dh permutation."""
    perm = np.empty(DH, dtype=np.int64)
    perm[0::2] = np.arange(DH // 2)
    perm[1::2] = np.arange(DH // 2) + DH // 2

    inv_freq = 1.0 / (ROPE_THETA ** (np.arange(0, DH, 2, dtype=np.float64) / DH))
    ang = np.arange(S, dtype=np.float64)[:, None] * inv_freq[None, :]  # [S, 32]
    cosv = np.cos(ang)
    sinv = np.sin(ang)
    C64 = np.empty((DH, S), dtype=np.float64)
    Ss64 = np.empty((DH, S), dtype=np.float64)
    for j in range(DH):
        C64[j] = cosv[:, j // 2]
        Ss64[j] = sinv[:, j // 2] * (1.0 if j % 2 == 0 else -1.0)
    return perm, C64, Ss64


def _host_inputs_fast(x, wq, wk, wv, wo):
    """Per-core inputs for the fast (causal) program, all fp16."""
    x2 = np.ascontiguousarray(x.reshape(S, D))
    xT = np.ascontiguousarray(x2.T)  # [D, S]
    # xS[p, sg*4096 + cd*512 + s] = xT[cd*128+p, sg*512+s]
    xT4 = xT.reshape(NCH_D, 128, NSG, 512)
    xS = np.ascontiguousarray(
        xT4.transpose(1, 2, 0, 3).reshape(128, NSG * NCH_D * 512)
    ).astype(np.float16)

    perm, C64, Ss64 = _rope_tables()
    cosT = np.ascontiguousarray(np.tile(C64, (2, 1))).astype(np.float16)
    sinTs = np.ascontiguousarray(np.tile(Ss64, (2, 1))).astype(np.float16)

    wq4 = wq.reshape(H, DH, D)
    wk4 = wk.reshape(HKV, DH, D)
    wv4 = wv.reshape(HKV, DH, D)

    def pack_w(w_c):
        """w_c [128 out-rows, D] -> [128 p, cd*128 + j] = w_c[j, cd*128+p]."""
        wT = np.ascontiguousarray(w_c.T)  # [D, 128]
        w3 = wT.reshape(NCH_D, 128, 128)  # [cd, p, j]
        return np.ascontiguousarray(
            w3.transpose(1, 0, 2).reshape(128, NCH_D * 128)).astype(np.float16)

    ins = []
    for c in range(NCORES):
        h0, h1 = 2 * c, 2 * c + 1
        g = h0 // (H // HKV)
        wq_c = np.concatenate([wq4[h0][perm], wq4[h1][perm]], axis=0)  # [128, D]
        wkv_c = np.concatenate([wk4[g][perm], wv4[g]], axis=0)         # [128, D]
        wo_c = wo[:, np.r_[h0 * DH:(h0 + 1) * DH, h1 * DH:(h1 + 1) * DH]]  # [D,128]
        ins.append({
            "xS": xS,
            "wqS": pack_w(wq_c),
            "wkvS": pack_w(wkv_c),
            "woT": np.ascontiguousarray(wo_c.T).astype(np.float16),
            "cosT": cosT,
            "sinTs": sinTs,
        })
    return ins


def _host_inputs_dense(x, wq, wk, wv, wo):
    x2 = np.ascontiguousarray(x.reshape(S, D))
    xT = np.ascontiguousarray(x2.T)

    perm, C64, Ss64 = _rope_tables()
    cosT = np.ascontiguousarray(np.tile(C64, (2, 1))).astype(np.float32)
    sinTs = np.ascontiguousarray(np.tile(Ss64, (2, 1))).astype(np.float32)

    wq4 = wq.reshape(H, DH, D)
    wk4 = wk.reshape(HKV, DH, D)
    wv4 = wv.reshape(HKV, DH, D)

    ins = []
    for c in range(NCORES):
        h0, h1 = 2 * c, 2 * c + 1
        g = h0 // (H // HKV)
        wq_c = np.concatenate([wq4[h0][perm], wq4[h1][perm]], axis=0)
        wk_c = np.concatenate([wk4[g][perm], wk4[g][perm]], axis=0)
        wo_c = wo[:, np.r_[h0 * DH:(h0 + 1) * DH, h1 * DH:(h1 + 1) * DH]]
        ins.append({
            "xT": xT,
            "wqT": np.ascontiguousarray(wq_c.T),
            "wkTd": np.ascontiguousarray(wk_c.T),
            "wvT": np.ascontiguousarray(wv4[g].T),
            "woT": np.ascontiguousarray(wo_c.T),
            "cosT": cosT,
            "sinTs": sinTs,
        })
    return ins


def _is_causal(mask):
    if mask.shape != (S, S):
        return False
    expected = np.where(np.tril(np.ones((S, S), dtype=bool)), np.float32(0.0),
                        np.float32(-1e9))
    return np.array_equal(mask, expected)


def _build(causal: bool):
    return _build_fast() if causal else _build_dense()


def run_cores(x, mask, wq, wk, wv, wo, **spmd_kwargs):
    """Compile (cached) + run on 8 cores; returns BassKernelResults."""
    from concourse.bass_utils import run_bass_kernel_spmd

    causal = _is_causal(np.asarray(mask))
    if causal not in _cache:
        _cache[causal] = _build(causal)
    nc = _cache[causal]

    if causal:
        ins = _host_inputs_fast(np.asarray(x), np.asarray(wq), np.asarray(wk),
                                np.asarray(wv), np.asarray(wo))
    else:
        ins = _host_inputs_dense(np.asarray(x), np.asarray(wq), np.asarray(wk),
                                 np.asarray(wv), np.asarray(wo))
        maskT = np.ascontiguousarray(np.asarray(mask).T)
        for d in ins:
            d["maskT"] = maskT
    res = run_bass_kernel_spmd(nc, ins, core_ids=list(range(NCORES)),
                               **spmd_kwargs)
    return res


def kernel(x, mask, wq, wk, wv, wo):
    res = run_cores(x, mask, wq, wk, wv, wo)
    acc = np.zeros((S, D), dtype=np.float64)
    for r in res.results:
        acc += r["out"].astype(np.float64)
    return acc.astype(np.float32).reshape(B, S, D)


# revision 58
# speedup vs baseline: 1.0189x; 1.0189x over previous
"""GQA causal attention (B=1, S=4096, D=1024, H=16, HKV=4, Dh=64, RoPE) on
8 Trainium2 NeuronCores.

Sharding: 8-way head parallelism. Core c owns query heads {2c, 2c+1} (which
share one KV head, g = c//2) and all 4096 query positions, so every core runs
the SAME program (one NEFF shared by all 8 cores) and only the weight shards
passed as inputs differ. Each core produces a partial output projection
[4096, 1024] (fp16, its heads' slice of wo); the host sums the 8 partials in
float64.

Device program (v2 — software-pipelined, fp16/bf16 datapath):
  All HBM traffic is fp16 and batched into few large DMAs (the DMA engines and
  the HWDGE descriptor generator are serial shared resources): x^T arrives as
  8 [128, 4096]-fp16 loads (one per 512-column group), tables/weights as one
  load each, the output as one [128, 1024]-fp16 store per 128-row block.

  Phase A (projections) is interleaved INTO phase B (attention) as "filler"
  units: the prologue projects column groups 0-1 (enough for q-tile 0), then
  groups {2,3}, {4,5}, {6,7} are loaded + projected + roped during q-tiles 0,
  1, 2 respectively, hiding all projection DMA/compute behind attention.
  Q and KV projections are merged where possible: K (64 rows) and V (64 rows)
  share one [128, 512] matmul chain per group. RoPE (rotate-half mapped to
  adjacent-pair shuffle via a host-side permutation of the weight rows) is
  applied by DVE reading the projection PSUM directly.

  Phase B per q-tile of 1024 columns, per head: per 128-key chunk,
  S^T = K^T_chunk^T @ Q^T (causal suffix only, fp16 in / fp32 PSUM out), exp
  on ScalarE reading PSUM (softmax is shift-invariant and scores are bounded
  << 88, so no row-max pass; fixed bias -10) written as bf16 (range!), the
  within-chunk upper triangle zeroed by gpsimd affine_select, then
  P@V accumulated in PSUM with a ones-column appended to V so row 64 collects
  the softmax denominator. Normalization: DVE copies the accumulator out,
  reciprocal on DVE, the per-column reciprocal row is broadcast across
  partitions by gpsimd (Pool engine — otherwise idle), DVE multiplies into
  the normalized ON buffer (fp16). The output projection of tile t
  (ON^T slices @ wo -> fp16 partials) is spread across tile t+1's chunks.

  Scheduling notes (learned against the TimelineSim cost model + real HW):
  PE is the bottleneck engine (~163us busy: scores+PV 113us, projections
  27us, out-proj 14us + overheads) with ScalarE exp second (~148us), so
  phase-A/oproj fillers are woven between chunks; each chunk's PV matmul is
  deferred one chunk (two at head switches) so PE never stalls on exp; junk
  transposes warm the PE p-state ramp during the DMA-bound prologue; the
  tail normalization is pipelined per 512-column segment. Hardware-found
  constraints: GPSIMD cannot touch PSUM, gpsimd partition_broadcast sources
  the TILE's partition 0 (not the AP's), reciprocal_approx_fast must read
  SBUF, and dma_start_transpose corrupts strided sub-tile destinations (PE
  transposes are used for V instead). Measured: 218us vs 282us for v1,
  rel err 4e-3 (fp16/bf16 rounding) vs 1.6e-3.

If the mask input is NOT the standard causal mask, the v1 dense fallback
program (all chunks, explicit mask add before exp, fp32 datapath) is compiled
instead: slower, still correct for any additive mask.
"""

import os

import numpy as np

B, S, D = 1, 4096, 1024
H, HKV, DH = 16, 4, 64
HPC = 2             # query heads per core
NCORES = 8
ROPE_THETA = 10000.0
QT_TILE = 1024      # q columns per attention tile
EXP_BIAS = -10.0    # shift inside exp; softmax-invariant, adds overflow headroom

NSG = S // 512      # 8 column groups
NCH_D = D // 128    # 8 contraction chunks for projections
NKCH = S // 128     # 32 key chunks
NQT = S // QT_TILE  # 4 q-tiles

_cache = {}


def _build_fast():
    """Causal-mask program (the fast path)."""
    import concourse.bass as bass
    import concourse.tile as tile
    from concourse import bacc, mybir
    from concourse.masks import make_identity

    f32 = mybir.dt.float32
    f16 = mybir.dt.float16
    bf16 = mybir.dt.bfloat16

    nc = bacc.Bacc(None, target_bir_lowering=False)

    # ---- DRAM I/O (all fp16) ----
    xS = nc.dram_tensor("xS", [128, NSG * NCH_D * 512], f16, kind="ExternalInput")
    wqS = nc.dram_tensor("wqS", [128, NCH_D * 128], f16, kind="ExternalInput")
    wkvS = nc.dram_tensor("wkvS", [128, NCH_D * 128], f16, kind="ExternalInput")
    woT = nc.dram_tensor("woT", [128, D], f16, kind="ExternalInput")
    cosT = nc.dram_tensor("cosT", [128, S], f16, kind="ExternalInput")
    sinTs = nc.dram_tensor("sinTs", [128, S], f16, kind="ExternalInput")
    out = nc.dram_tensor("out", [S, D], f16, kind="ExternalOutput")
    dbg = bool(os.environ.get("KDBG"))
    if dbg:
        dbg_qtr = nc.dram_tensor("dbg_qtr", [128, S], f16, kind="ExternalOutput")
        dbg_ktr = nc.dram_tensor("dbg_ktr", [128, S], f16, kind="ExternalOutput")
        dbg_on = nc.dram_tensor("dbg_on", [128, S], f16, kind="ExternalOutput")
        dbg_vp = nc.dram_tensor("dbg_vp", [128, NKCH * (64 + DH)], bf16,
                                kind="ExternalOutput")
        dbg_den = nc.dram_tensor("dbg_den", [NQT * HPC, QT_TILE], f32,
                                 kind="ExternalOutput")

    with tile.TileContext(nc) as tc:
        with tc.tile_pool(name="const", bufs=1) as cpool, \
             tc.tile_pool(name="xs", bufs=2) as xs_pool, \
             tc.tile_pool(name="rtmp", bufs=2) as rtmp, \
             tc.tile_pool(name="vtt", bufs=2) as vtt_pool, \
             tc.tile_pool(name="esb", bufs=1) as e_pool, \
             tc.tile_pool(name="osb", bufs=2) as ot_pool, \
             tc.tile_pool(name="obst", bufs=6 if os.environ.get("KOB6") else 4) as ob_pool, \
             tc.tile_pool(name="sps", bufs=1, space="PSUM") as s_ps_pool, \
             tc.tile_pool(name="ops", bufs=1, space="PSUM") as o_ps_pool, \
             tc.tile_pool(name="msc", bufs=1, space="PSUM") as misc_pool:

            # ---- resident constants / accumulators ----
            wq_sb = cpool.tile([128, NCH_D * 128], f16)
            wkv_sb = cpool.tile([128, NCH_D * 128], f16)
            wo_sb = cpool.tile([128, D], f16)
            cos_sb = cpool.tile([128, S], f16)
            sin_sb = cpool.tile([128, S], f16)
            QTr = cpool.tile([128, S], f16)        # rope(Q)^T, rows 0-63 h0, 64-127 h1
            KTr = cpool.tile([128, S], f16)        # rope(K)^T, duplicated in both halves
            VOFF = 64
            Vp = cpool.tile([128, NKCH, VOFF + DH], bf16)  # ones, zero pad, V
            ON = cpool.tile([128, S], f16)         # normalized O^T
            ident = cpool.tile([DH, DH], f32)
            biasc = cpool.tile([128, 1], f32)
            ones_row = cpool.tile([128, DH], f32)

            make_identity(nc, ident[:, :])
            nc.vector.memset(biasc, float(EXP_BIAS))
            nc.vector.memset(ones_row, 1.0)
            # ones column at index 0 (V at VOFF..VOFF+DH): the denominator
            # lands in accumulator row 0 = tile partition 0, which is exactly
            # what gpsimd partition_broadcast sources; the numerators start at
            # partition VOFF=32 (engine partition bases must be 32-aligned)
            nc.vector.memset(Vp[:, :, 0:1], 1.0)
            nc.vector.memset(Vp[:, :, 1:VOFF], 0.0)

            # ---- constant DMAs (scalar queue; ordered for earliest phase B
            #      start: xs0, the first table halves, then the rest) ----
            HS = 1024
            nc.scalar.dma_start(out=wq_sb, in_=wqS[:, :])

            def dma_xs(g):
                xs = xs_pool.tile([128, NCH_D * 512], f16, tag="xs",
                                  name=f"xs_{g}")
                nc.sync.dma_start(out=xs, in_=xS[:, g * NCH_D * 512:
                                                (g + 1) * NCH_D * 512])
                return xs

            xs_tiles = {}
            xs_tiles[0] = dma_xs(0)
            nc.scalar.dma_start(out=wkv_sb, in_=wkvS[:, :])
            nc.scalar.dma_start(out=cos_sb[:, 0:HS], in_=cosT[:, 0:HS])
            nc.scalar.dma_start(out=sin_sb[:, 0:HS], in_=sinTs[:, 0:HS])
            xs_tiles[1] = dma_xs(1)
            nc.scalar.dma_start(out=cos_sb[:, HS:S], in_=cosT[:, HS:S])
            nc.scalar.dma_start(out=sin_sb[:, HS:S], in_=sinTs[:, HS:S])
            nc.scalar.dma_start(out=wo_sb, in_=woT[:, :])

            # misc PSUM: two 1-bank slots, round-robin for all filler matmuls
            misc_state = [0]

            def misc_tile(name):
                tag = "mt"[misc_state[0]]
                misc_state[0] ^= 1
                return misc_pool.tile([128, 512], f32, tag=tag, name=name)

            # PE p-state warmup: the tensor engine only reaches full clock
            # after 3us of continuous execution; burn junk transposes during
            # the (DMA-bound) prologue so the first projections run at speed.
            for w in range(3):
                wt = misc_tile(f"warm{w}")
                for j in range(8):
                    nc.tensor.transpose(wt[0:64, j * 64:(j + 1) * 64],
                                        ones_row[0:64, :],
                                        ident[:, :])

            # ---- phase A unit generators (per 512-column group) ----
            SHUF = [i ^ 1 for i in range(32)]

            def rope_from_psum(ps_ap, sb_out_ap, scols, nrow, dup_out=None):
                m1 = rtmp.tile([128, 512], f16, tag="rope_m1")
                m2 = rtmp.tile([128, 512], f16, tag="rope_m2")
                sh = rtmp.tile([128, 512], f16, tag="rope_sh")
                nc.vector.tensor_mul(m1[0:nrow, :], ps_ap, cos_sb[0:nrow, scols])
                nc.vector.tensor_mul(m2[0:nrow, :], ps_ap, sin_sb[0:nrow, scols])
                nc.vector.stream_shuffle(sh[0:nrow, :], m2[0:nrow, :], SHUF)
                nc.vector.tensor_add(sb_out_ap, m1[0:nrow, :], sh[0:nrow, :])
                if dup_out is not None:
                    nc.vector.tensor_add(dup_out, m1[0:nrow, :], sh[0:nrow, :])

            def group_units(g):
                """Return list of emission closures for phase-A group g
                (xs DMA must already have been issued; xs_tiles[g] set)."""
                scols = bass.ds(g * 512, 512)
                st = {}

                def u_qmm():
                    st["qt"] = misc_tile(f"qt_{g}")
                    xs = xs_tiles[g]
                    for cd in range(NCH_D):
                        nc.tensor.matmul(st["qt"][:, :],
                                         wq_sb[:, cd * 128:(cd + 1) * 128],
                                         xs[:, cd * 512:(cd + 1) * 512],
                                         start=(cd == 0), stop=(cd == NCH_D - 1))

                def u_qrope():
                    rope_from_psum(st["qt"][:, :], QTr[:, scols], scols, 128)

                def u_kvmm():
                    st["kv"] = misc_tile(f"kv_{g}")
                    xs = xs_tiles[g]
                    for cd in range(NCH_D):
                        nc.tensor.matmul(st["kv"][:, :],
                                         wkv_sb[:, cd * 128:(cd + 1) * 128],
                                         xs[:, cd * 512:(cd + 1) * 512],
                                         start=(cd == 0), stop=(cd == NCH_D - 1))

                def u_krope():
                    rope_from_psum(st["kv"][0:64, :], KTr[0:64, scols], scols,
                                   64, dup_out=KTr[64:128, scols])

                def u_vcopy():
                    vt = vtt_pool.tile([64, 512], f32, tag="vt")
                    st["vt"] = vt
                    nc.vector.tensor_copy(vt, st["kv"][64:128, :])

                def u_vtrans():
                    tr = misc_tile(f"tr_{g}")
                    for j in range(4):
                        nc.tensor.transpose(tr[:, j * 64:(j + 1) * 64],
                                            st["vt"][:, j * 128:(j + 1) * 128],
                                            ident[:, :])
                    nc.vector.tensor_copy(
                        Vp[:, g * 4:(g + 1) * 4, VOFF:VOFF + DH],
                        tr[:, 0:4 * DH].rearrange("p (j d) -> p j d", j=4))

                return [u_qmm, u_qrope, u_kvmm, u_krope, u_vcopy, u_vtrans]

            # ---- phase B helpers ----
            # (GPSIMD cannot read PSUM, so staging copies go DVE / ScalarE)
            COPY_ENGINES = [
                lambda o, i: nc.vector.tensor_copy(o, i),
                lambda o, i: nc.scalar.copy(o, i),
            ]

            def emit_oproj(t, qsub, dseg, ob_state, psum_tile=None,
                           eng_idx=None):
                """One output-projection unit: [128 q, 512 d] partial."""
                qg = t * (QT_TILE // 128) + qsub
                if dseg == 0:
                    ob_state[qg] = ob_pool.tile([128, D], f16, tag="ob",
                                                name=f"ob_{qg}")
                op = psum_tile if psum_tile is not None \
                    else misc_tile(f"op_{qg}_{dseg}")
                nc.tensor.matmul(
                    op[:, 0:512],
                    ON[:, qg * 128:(qg + 1) * 128],
                    wo_sb[:, dseg * 512:(dseg + 1) * 512],
                    start=True, stop=True)
                if eng_idx is None:
                    eng_idx = 1 if (os.environ.get("KACTCP")
                                    and t < 2 and (qg + dseg) % 2 == 0) else 0
                cp = COPY_ENGINES[eng_idx]
                cp(ob_state[qg][:, dseg * 512:(dseg + 1) * 512], op[:, 0:512])
                if dseg == (D // 512) - 1:
                    nc.sync.dma_start(
                        out=out[qg * 128:(qg + 1) * 128, :],
                        in_=ob_state[qg])

            def emit_norm(t, h, o_ps, seg_hook=None, direct=False):
                """Normalize head h of tile t: ON[64h:64h+64, tile cols] =
                o_ps numerators * (1/denominator row). direct=True (last
                head only) skips the PSUM->SBUF staging copy — o_ps is not
                needed for a next head, so DVE reads it in place."""
                q0 = t * QT_TILE
                rc = ot_pool.tile([VOFF + DH, QT_TILE], f32, tag="rc",
                                  name=f"rc_{t}_{h}")
                ot = ot_pool.tile([VOFF + DH, QT_TILE], f32, tag="ot",
                                  name=f"ot_{t}_{h}")
                if direct:
                    # tail head: per-segment pipeline so bcast/mul/oproj of
                    # segment 0 overlap the copy/recip of segment 1
                    segs = []
                    for seg in range(QT_TILE // 512):
                        cs = bass.ds(seg * 512, 512)
                        nc.vector.tensor_copy(ot[:, cs], o_ps[:, cs])
                        nc.vector.reciprocal_approx_fast(rc[:, cs],
                                                         ot[:, cs])
                        segs.append((seg, cs))
                else:
                    nc.vector.tensor_copy(ot, o_ps[:, :])
                    nc.vector.reciprocal_approx_fast(rc, ot[:, :])
                    segs = [(seg, bass.ds(seg * 512, 512))
                            for seg in range(QT_TILE // 512)]
                if dbg:
                    nc.sync.dma_start(out=dbg_den[t * HPC + h:t * HPC + h + 1, :],
                                      in_=ot[0:1, :])
                for seg, cs in segs:
                    # 1/denominator sits at rc row 0 == tile partition 0, the
                    # row partition_broadcast replicates; bc row 0 is then
                    # dropped so the mul's inputs share partition base 1
                    bc = ot_pool.tile([VOFF + DH, 512], f32, tag=f"bc{seg}",
                                      name=f"bc_{t}_{h}_{seg}")
                    nc.gpsimd.partition_broadcast(bc[:, :], rc[0:1, cs],
                                                  channels=VOFF + DH)
                    nc.vector.tensor_mul(
                        ON[64 * h:64 * h + 64,
                           q0 + seg * 512:q0 + (seg + 1) * 512],
                        ot[VOFF:VOFF + DH, seg * 512:(seg + 1) * 512],
                        bc[VOFF:VOFF + DH, :])
                    if seg_hook is not None:
                        seg_hook(seg)

            # Global deferred-PV stream state: PE order per chunk is
            # [score(c), PV(c-1)], so PE never waits on exp(c) — it always has
            # the previous chunk's PV (whose exp finished during score(c)).
            # A head's FIRST PV is deferred one extra chunk: it write-after-
            # read depends on the previous head's accumulator staging copy
            # (DVE), which needs the extra slack.
            pv_q = []      # [t, h, c, qs, e_sb, first, lastc, age]
            pv_o = [None]  # current PSUM accumulator

            def pump_pv(force=False, norm_seg_hook=None):
                while pv_q:
                    t, h, c, qs, e_sb, first, lastc, age = pv_q[0]
                    if not (force or age >= 2 or (age >= 1 and not first)):
                        return
                    pv_q.pop(0)
                    if first:
                        pv_o[0] = o_ps_pool.tile(
                            [VOFF + DH, QT_TILE], f32, tag="oacc",
                            name=f"ops_{t}_{h}")
                    for lo, hi in ((qs, 512), (max(qs, 512), QT_TILE)):
                        if lo >= hi:
                            continue
                        cs = bass.ds(lo, hi - lo)
                        nc.tensor.matmul(
                            pv_o[0][:, cs], Vp[:, c, 0:VOFF + DH],
                            e_sb[:, cs],
                            start=first, stop=lastc)
                    if lastc:
                        last = (t == NQT - 1 and h == HPC - 1)
                        emit_norm(t, h, pv_o[0], seg_hook=norm_seg_hook,
                                  direct=last)

            def chunk_units(t, h, nch):
                """Emission closures for all key-chunks of (tile t, head h).
                Chunks are emitted with the short diagonal chunks interleaved
                among the full-height ones (PV accumulation is commutative),
                so the short chunks' dependency latency hides under the long
                chunks' exp time."""
                q0 = t * QT_TILE
                full = list(range(0, 8 * t))
                diag = list(range(8 * t, nch))
                order = []
                if full and os.environ.get("KILV"):
                    r = max(1, len(full) // len(diag))
                    di = 0
                    for k, c in enumerate(full):
                        order.append(c)
                        if k % r == r - 1 and di < len(diag):
                            order.append(diag[di])
                            di += 1
                    order += diag[di:]
                else:
                    order = full + diag

                def mk(ci, c, first, lastc):
                    def u():
                        qs = max(0, c * 128 - q0)
                        s_ps = s_ps_pool.tile([128, QT_TILE], f32,
                                              tag=f"s{ci % 2}",
                                              name=f"s_{t}_{h}_{ci}")
                        for lo, hi in ((qs, 512), (max(qs, 512), QT_TILE)):
                            if lo >= hi:
                                continue
                            nc.tensor.matmul(
                                s_ps[:, bass.ds(lo, hi - lo)],
                                KTr[64 * h:64 * h + 64, c * 128:(c + 1) * 128],
                                QTr[64 * h:64 * h + 64, q0 + lo:q0 + hi],
                                start=True, stop=True)
                        for p in pv_q:
                            p[7] += 1
                        pump_pv()
                        e_sb = e_pool.tile([128, QT_TILE], bf16,
                                           tag=f"e{ci % (4 if os.environ.get('KE4') else 3)}",
                                           name=f"e_{t}_{h}_{ci}")
                        nc.scalar.activation(
                            e_sb[:, qs:QT_TILE], s_ps[:, qs:QT_TILE],
                            mybir.ActivationFunctionType.Exp,
                            bias=biasc[:, :], scale=1.0)
                        if c * 128 >= q0:
                            nc.gpsimd.affine_select(
                                out=e_sb[:, qs:qs + 128],
                                in_=e_sb[:, qs:qs + 128],
                                pattern=[[1, 128]],
                                compare_op=mybir.AluOpType.is_ge,
                                fill=0.0, base=0, channel_multiplier=-1)
                        pv_q.append([t, h, c, qs, e_sb, first, lastc, 0])
                    return u

                return [mk(ci, c, ci == 0, ci == nch - 1)
                        for ci, c in enumerate(order)]

            # ---- prologue: just enough for tile-0 chunk 0 — group 0 fully,
            #      group 1's Q projection + rope. Group 1's K/V (first needed
            #      by chunk 4) moves into tile 0's first filler slots. ----
            g0u = group_units(0)
            g1u = group_units(1)
            for u in (g0u[0], g0u[1],            # qt0, ropeQ0
                      g0u[2], g0u[3], g0u[4],    # kv0, ropeK0, vcopy0
                      g1u[0], g1u[1],            # qt1, ropeQ1
                      g0u[5]):                   # tr0 (dma)
                u()
            pre_fillers = [g1u[2], g1u[3], g1u[4], g1u[5]]

            # ---- main loop: tiles with woven fillers ----
            # group g is loaded+projected during tile (g-2)//2 wait... groups
            # 2..7 are spread {t0: g2,g3-dma, t1: g3,g4,g5-dma, ...} — see
            # TILE_GROUPS; dma for group g is issued right after group (g-2)'s
            # last unit so its xs-pool slot is free and the load hides.
            TILE_GROUPS = {0: [2, 3], 1: [4, 5], 2: [6, 7], 3: []}
            ob_state = {}
            pending_op = []   # oproj args from previous tile

            xs_tiles[2] = dma_xs(2)
            xs_tiles[3] = dma_xs(3)
            for t in range(NQT):
                nch = (t + 1) * (QT_TILE // 128)

                fillers = []
                if t == 0:
                    fillers += pre_fillers
                ops = [lambda tp=tp, q=qsub, d=dseg:
                       emit_oproj(tp, q, d, ob_state)
                       for (tp, qsub, dseg) in pending_op]
                pending_op = []
                # a couple of (dependency-free) oproj units first, then the
                # phase-A group units (their xs arrived a tile ago), with the
                # next groups' xs loads issued as their slots free up
                fillers += ops[:2]
                ops = ops[2:]
                for g in TILE_GROUPS[t]:
                    fillers += group_units(g)
                    if 4 <= g + 2 <= 7:
                        fillers.append(lambda g2=g + 2: xs_tiles.__setitem__(
                            g2, dma_xs(g2)))
                    nops = 6 if g % 2 == 0 else len(ops)
                    fillers += ops[:nops]
                    ops = ops[nops:]
                fillers += ops

                chunks = chunk_units(t, 0, nch) + chunk_units(t, 1, nch)
                n = len(chunks)
                m = len(fillers)
                lead = 1
                span = max(1, n - lead - 2)
                # keep fillers out of the PE queue around head switches so the
                # next head's first score matmul issues immediately (ACT would
                # otherwise stall behind a filler projection)
                nofill = set()
                fi = 0
                for i, ce in enumerate(chunks):
                    ce()
                    if i in nofill:
                        continue
                    tgt = 0 if i < lead else min(
                        m, (m * (i - lead + 1) + span - 1) // span)
                    while fi < tgt:
                        fillers[fi]()
                        fi += 1
                while fi < m:
                    fillers[fi]()
                    fi += 1

                pending_op = [(t, qsub, dseg)
                              for qsub in range(QT_TILE // 128)
                              for dseg in range(D // 512)]

            # ---- tail: flush last PV + norm, then the last tile's output
            #      projection with deep PSUM rotation (s banks are free now)
            #      and copies spread across DVE/Pool/ScalarE (all idle) ----
            tail_ops = pending_op
            tail_i = [0]

            def tail_psum(name):
                i = tail_i[0]
                if i % 4 < 2:
                    return misc_tile(name)
                return s_ps_pool.tile([128, QT_TILE], f32,
                                      tag=f"s{i % 2}", name=name)

            def tail_emit(seg):
                # oproj units whose q-block lies in this 512-col segment
                for (tp, qsub, dseg) in tail_ops:
                    if qsub // 4 != seg:
                        continue
                    emit_oproj(tp, qsub, dseg, ob_state,
                               psum_tile=tail_psum(f"top_{qsub}_{dseg}"),
                               eng_idx=tail_i[0] % 2)
                    tail_i[0] += 1

            pump_pv(force=True, norm_seg_hook=tail_emit)

            if dbg:
                nc.sync.dma_start(out=dbg_qtr[:, :], in_=QTr[:, :])
                nc.sync.dma_start(out=dbg_ktr[:, :], in_=KTr[:, :])
                nc.sync.dma_start(out=dbg_on[:, :], in_=ON[:, :])
                nc.sync.dma_start(
                    out=dbg_vp[:, :],
                    in_=Vp[:, :, :].rearrange("p a b -> p (a b)"))

    nc.compile()
    return nc


def _build_dense():
    """Fallback for a non-causal additive mask (v1 program, fp32 datapath)."""
    import concourse.bass as bass
    import concourse.tile as tile
    from concourse import bacc, mybir
    from concourse.masks import make_identity

    f32 = mybir.dt.float32
    f16 = mybir.dt.float16
    f32r = mybir.dt.float32r

    nc = bacc.Bacc(None, target_bir_lowering=False)

    xT = nc.dram_tensor("xT", [D, S], f32r, kind="ExternalInput")
    wqT = nc.dram_tensor("wqT", [D, 128], f32r, kind="ExternalInput")
    wkTd = nc.dram_tensor("wkTd", [D, 128], f32r, kind="ExternalInput")
    wvT = nc.dram_tensor("wvT", [D, DH], f32r, kind="ExternalInput")
    woT = nc.dram_tensor("woT", [128, D], f32r, kind="ExternalInput")
    cosT = nc.dram_tensor("cosT", [128, S], f32, kind="ExternalInput")
    sinTs = nc.dram_tensor("sinTs", [128, S], f32, kind="ExternalInput")
    maskT = nc.dram_tensor("maskT", [S, S], f32, kind="ExternalInput")
    out = nc.dram_tensor("out", [S, D], f16, kind="ExternalOutput")

    from contextlib import ExitStack
    with tile.TileContext(nc) as tc, ExitStack() as phase_a:
        with tc.tile_pool(name="const", bufs=1) as cpool, \
             tc.tile_pool(name="xs", bufs=4) as xs_pool, \
             tc.tile_pool(name="rtmp", bufs=2) as rtmp, \
             tc.tile_pool(name="vtt", bufs=2) as vtt_pool, \
             tc.tile_pool(name="esb", bufs=2) as e_pool, \
             tc.tile_pool(name="osb", bufs=2) as ot_pool, \
             tc.tile_pool(name="mtile", bufs=2) as m_pool:
            prj_ps = phase_a.enter_context(tc.tile_pool(name="prj", bufs=2, space="PSUM"))
            trp_ps = phase_a.enter_context(tc.tile_pool(name="trp", bufs=2, space="PSUM"))

            wq_sb = cpool.tile([128, NCH_D, 128], f32r)
            wk_sb = cpool.tile([128, NCH_D, 128], f32r)
            wv_sb = cpool.tile([128, NCH_D, DH], f32r)
            wo_sb = cpool.tile([128, D], f32r)
            cos_sb = cpool.tile([128, S], f32)
            sin_sb = cpool.tile([128, S], f32)
            QTr = cpool.tile([128, S], f32r)
            KTr = cpool.tile([128, S], f32r)
            Vp = cpool.tile([128, NKCH, DH + 1], f32r)
            ON = cpool.tile([128, S], f32r)
            ident = cpool.tile([DH, DH], f32)
            ones_row = cpool.tile([128, DH], f32)
            biasc = cpool.tile([128, 1], f32)

            for cd in range(NCH_D):
                nc.scalar.dma_start(out=wq_sb[:, cd, :], in_=wqT[cd * 128:(cd + 1) * 128, :])
                nc.scalar.dma_start(out=wk_sb[:, cd, :], in_=wkTd[cd * 128:(cd + 1) * 128, :])
                nc.scalar.dma_start(out=wv_sb[:, cd, :], in_=wvT[cd * 128:(cd + 1) * 128, :])
            for sg in range(NSG):
                sl = bass.ds(sg * 512, 512)
                nc.scalar.dma_start(out=cos_sb[:, sl], in_=cosT[:, sg * 512:(sg + 1) * 512])
                nc.scalar.dma_start(out=sin_sb[:, sl], in_=sinTs[:, sg * 512:(sg + 1) * 512])
            nc.scalar.dma_start(out=wo_sb, in_=woT[:, :])
            make_identity(nc, ident[:, :])
            nc.vector.memset(ones_row, 1.0)
            nc.vector.memset(biasc, float(EXP_BIAS))
            nc.vector.memset(Vp[:, :, DH:DH + 1].bitcast(f32), 1.0)

            def rope_from_psum(ps_ap, sb_out_ap, scols, width):
                m1 = rtmp.tile([128, 512], f32, tag="rope_m1")
                m2 = rtmp.tile([128, 512], f32, tag="rope_m2")
                sh = rtmp.tile([128, 512], f32, tag="rope_sh")
                nc.vector.tensor_mul(m1[:, :width], ps_ap, cos_sb[:, scols])
                nc.vector.tensor_mul(m2[:, :width], ps_ap, sin_sb[:, scols])
                nc.vector.stream_shuffle(sh[:, :width], m2[:, :width],
                                         [i ^ 1 for i in range(32)])
                nc.vector.tensor_add(sb_out_ap, m1[:, :width], sh[:, :width])

            for sg in range(NSG):
                scols = bass.ds(sg * 512, 512)
                qt_ps = prj_ps.tile([128, 512], f32, tag="qt")
                kt_ps = prj_ps.tile([128, 512], f32, tag="kt")
                vt_ps = prj_ps.tile([DH, 512], f32, tag="vt")
                for cd in range(NCH_D):
                    xsl = xs_pool.tile([128, 512], f32r)
                    nc.sync.dma_start(out=xsl, in_=xT[cd * 128:(cd + 1) * 128,
                                                      sg * 512:(sg + 1) * 512])
                    st = (cd == 0)
                    sp = (cd == NCH_D - 1)
                    nc.tensor.matmul(qt_ps[:, :], wq_sb[:, cd, :], xsl,
                                     start=st, stop=sp)
                    nc.tensor.matmul(kt_ps[:, :], wk_sb[:, cd, :], xsl,
                                     start=st, stop=sp)
                    nc.tensor.matmul(vt_ps[:, :], wv_sb[:, cd, :], xsl,
                                     start=st, stop=sp)
                rope_from_psum(qt_ps[:, :], QTr[:, scols], scols, 512)
                rope_from_psum(kt_ps[:, :], KTr[:, scols], scols, 512)
                vt_sb = vtt_pool.tile([DH, 512], f32)
                nc.scalar.copy(vt_sb, vt_ps[:, :])
                for j in range(4):
                    kc = sg * 4 + j
                    tr = trp_ps.tile([128, DH], f32)
                    nc.tensor.transpose(tr[:, :], vt_sb[:, j * 128:(j + 1) * 128],
                                        ident[:, :])
                    nc.vector.tensor_copy(Vp[:, kc, 0:DH], tr[:, :])

            phase_a.close()
            phase_b = ExitStack()
            s_ps_pool = phase_b.enter_context(tc.tile_pool(name="sps", bufs=1, space="PSUM"))
            o_ps_pool = phase_b.enter_context(tc.tile_pool(name="ops", bufs=1, space="PSUM"))

            def emit_oproj(qsub, dseg):
                op = o_ps_pool.tile([128, 512], f32, tag=f"op{dseg}",
                                    name=f"op_{qsub}_{dseg}")
                nc.tensor.matmul(
                    op[:, :],
                    ON[:, qsub * 128:(qsub + 1) * 128],
                    wo_sb[:, dseg * 512:(dseg + 1) * 512],
                    start=True, stop=True)
                ob = m_pool.tile([128, 512], f16, tag="ostage")
                nc.vector.tensor_copy(ob, op[:, :])
                nc.sync.dma_start(
                    out=out[qsub * 128:(qsub + 1) * 128,
                            dseg * 512:(dseg + 1) * 512],
                    in_=ob)

            pending = []

            def emit_norm(tn, hn, o_psn):
                ot = ot_pool.tile([DH + 1, QT_TILE], f32, tag="ot",
                                  name=f"ot_{tn}_{hn}")
                nc.vector.tensor_copy(ot, o_psn[:, :])
                q0n = tn * QT_TILE
                rc = ot_pool.tile([DH + 1, QT_TILE], f32, tag="rc",
                                  name=f"rc_{tn}_{hn}")
                nc.vector.reciprocal_approx_fast(rc, ot[:, :])
                for seg in range(QT_TILE // 512):
                    cs = bass.ds(seg * 512, 512)
                    rbseg = o_ps_pool.tile([128, 512], f32, tag=f"op{seg}",
                                           name=f"rb_{tn}_{hn}_{seg}")
                    nc.tensor.matmul(rbseg[0:DH, :],
                                     ones_row[DH:DH + 1, :],
                                     rc[DH:DH + 1, cs],
                                     start=True, stop=True)
                    nc.vector.tensor_mul(
                        ON[64 * hn:64 * hn + 64,
                           q0n + seg * 512:q0n + (seg + 1) * 512],
                        ot[0:DH, seg * 512:(seg + 1) * 512], rbseg[0:DH, :])

            for t in range(NQT):
                q0 = t * QT_TILE
                nch = NKCH
                for h in range(HPC):
                    o_ps = o_ps_pool.tile([DH + 1, QT_TILE], f32, tag="oacc",
                                          name=f"ops_{t}_{h}")
                    for ci in range(nch):
                        c = ci
                        s_ps = s_ps_pool.tile([128, QT_TILE], f32,
                                              tag=f"s{ci % 2}",
                                              name=f"s_{t}_{h}_{ci}")
                        lhs = KTr[64 * h:64 * h + 64, c * 128:(c + 1) * 128]
                        for lo, hi in ((0, 512), (512, QT_TILE)):
                            nc.tensor.matmul(
                                s_ps[:, bass.ds(lo, hi - lo)], lhs,
                                QTr[64 * h:64 * h + 64, q0 + lo:q0 + hi],
                                start=True, stop=True)
                        e_sb = e_pool.tile([128, QT_TILE], f32r, tag=f"e{h}",
                                           name=f"e_{t}_{h}_{ci}")
                        sm = m_pool.tile([128, QT_TILE], f32, tag="mask")
                        nc.sync.dma_start(
                            out=sm, in_=maskT[c * 128:(c + 1) * 128,
                                              q0:q0 + QT_TILE])
                        sms = m_pool.tile([128, QT_TILE], f32, tag="masked")
                        nc.vector.tensor_add(sms, s_ps[:, :], sm)
                        nc.scalar.activation(
                            e_sb[:, :], sms,
                            mybir.ActivationFunctionType.Exp,
                            bias=biasc[:, :], scale=1.0)
                        for lo, hi in ((0, 512), (512, QT_TILE)):
                            cs = bass.ds(lo, hi - lo)
                            nc.tensor.matmul(
                                o_ps[:, cs], Vp[:, c, :],
                                e_sb[:, cs],
                                start=(c == 0), stop=(c == nch - 1))
                        if pending:
                            emit_oproj(*pending.pop(0))
                    emit_norm(t, h, o_ps)
                for j in range(QT_TILE // 128):
                    for dseg in range(D // 512):
                        pending.append((t * (QT_TILE // 128) + j, dseg))
            while pending:
                emit_oproj(*pending.pop(0))
            phase_b.close()

    nc.compile()
    return nc


def _rope_tables():
    """Pair-interleaved rope tables + the dh permutation."""
    perm = np.empty(DH, dtype=np.int64)
    perm[0::2] = np.arange(DH // 2)
    perm[1::2] = np.arange(DH // 2) + DH // 2

    inv_freq = 1.0 / (ROPE_THETA ** (np.arange(0, DH, 2, dtype=np.float64) / DH))
    ang = np.arange(S, dtype=np.float64)[:, None] * inv_freq[None, :]  # [S, 32]
    cosv = np.cos(ang)
    sinv = np.sin(ang)
    C64 = np.empty((DH, S), dtype=np.float64)
    Ss64 = np.empty((DH, S), dtype=np.float64)
    for j in range(DH):
        C64[j] = cosv[:, j // 2]
        Ss64[j] = sinv[:, j // 2] * (1.0 if j % 2 == 0 else -1.0)
    return perm, C64, Ss64


def _host_inputs_fast(x, wq, wk, wv, wo):
    """Per-core inputs for the fast (causal) program, all fp16."""
    x2 = np.ascontiguousarray(x.reshape(S, D))
    xT = np.ascontiguousarray(x2.T)  # [D, S]
    # xS[p, sg*4096 + cd*512 + s] = xT[cd*128+p, sg*512+s]
    xT4 = xT.reshape(NCH_D, 128, NSG, 512)
    xS = np.ascontiguousarray(
        xT4.transpose(1, 2, 0, 3).reshape(128, NSG * NCH_D * 512)
    ).astype(np.float16)

    perm, C64, Ss64 = _rope_tables()
    cosT = np.ascontiguousarray(np.tile(C64, (2, 1))).astype(np.float16)
    sinTs = np.ascontiguousarray(np.tile(Ss64, (2, 1))).astype(np.float16)

    wq4 = wq.reshape(H, DH, D)
    wk4 = wk.reshape(HKV, DH, D)
    wv4 = wv.reshape(HKV, DH, D)

    def pack_w(w_c):
        """w_c [128 out-rows, D] -> [128 p, cd*128 + j] = w_c[j, cd*128+p]."""
        wT = np.ascontiguousarray(w_c.T)  # [D, 128]
        w3 = wT.reshape(NCH_D, 128, 128)  # [cd, p, j]
        return np.ascontiguousarray(
            w3.transpose(1, 0, 2).reshape(128, NCH_D * 128)).astype(np.float16)

    ins = []
    for c in range(NCORES):
        h0, h1 = 2 * c, 2 * c + 1
        g = h0 // (H // HKV)
        wq_c = np.concatenate([wq4[h0][perm], wq4[h1][perm]], axis=0)  # [128, D]
        wkv_c = np.concatenate([wk4[g][perm], wv4[g]], axis=0)         # [128, D]
        wo_c = wo[:, np.r_[h0 * DH:(h0 + 1) * DH, h1 * DH:(h1 + 1) * DH]]  # [D,128]
        ins.append({
            "xS": xS,
            "wqS": pack_w(wq_c),
            "wkvS": pack_w(wkv_c),
            "woT": np.ascontiguousarray(wo_c.T).astype(np.float16),
            "cosT": cosT,
            "sinTs": sinTs,
        })
    return ins


def _host_inputs_dense(x, wq, wk, wv, wo):
    x2 = np.ascontiguousarray(x.reshape(S, D))
    xT = np.ascontiguousarray(x2.T)

    perm, C64, Ss64 = _rope_tables()
    cosT = np.ascontiguousarray(np.tile(C64, (2, 1))).astype(np.float32)
    sinTs = np.ascontiguousarray(np.tile(Ss64, (2, 1))).astype(np.float32)

    wq4 = wq.reshape(H, DH, D)
    wk4 = wk.reshape(HKV, DH, D)
    wv4 = wv.reshape(HKV, DH, D)

    ins = []
    for c in range(NCORES):
        h0, h1 = 2 * c, 2 * c + 1
        g = h0 // (H // HKV)
        wq_c = np.concatenate([wq4[h0][perm], wq4[h1][perm]], axis=0)
        wk_c = np.concatenate([wk4[g][perm], wk4[g][perm]], axis=0)
        wo_c = wo[:, np.r_[h0 * DH:(h0 + 1) * DH, h1 * DH:(h1 + 1) * DH]]
        ins.append({
            "xT": xT,
            "wqT": np.ascontiguousarray(wq_c.T),
            "wkTd": np.ascontiguousarray(wk_c.T),
            "wvT": np.ascontiguousarray(wv4[g].T),
            "woT": np.ascontiguousarray(wo_c.T),
            "cosT": cosT,
            "sinTs": sinTs,
        })
    return ins


def _is_causal(mask):
    if mask.shape != (S, S):
        return False
    expected = np.where(np.tril(np.ones((S, S), dtype=bool)), np.float32(0.0),
                        np.float32(-1e9))
    return np.array_equal(mask, expected)


def _build(causal: bool):
    return _build_fast() if causal else _build_dense()


def run_cores(x, mask, wq, wk, wv, wo, **spmd_kwargs):
    """Compile (cached) + run on 8 cores; returns BassKernelResults."""
    from concourse.bass_utils import run_bass_kernel_spmd

    causal = _is_causal(np.asarray(mask))
    if causal not in _cache:
        _cache[causal] = _build(causal)
    nc = _cache[causal]

    if causal:
        ins = _host_inputs_fast(np.asarray(x), np.asarray(wq), np.asarray(wk),
                                np.asarray(wv), np.asarray(wo))
    else:
        ins = _host_inputs_dense(np.asarray(x), np.asarray(wq), np.asarray(wk),
                                 np.asarray(wv), np.asarray(wo))
        maskT = np.ascontiguousarray(np.asarray(mask).T)
        for d in ins:
            d["maskT"] = maskT
    res = run_bass_kernel_spmd(nc, ins, core_ids=list(range(NCORES)),
                               **spmd_kwargs)
    return res


def kernel(x, mask, wq, wk, wv, wo):
    res = run_cores(x, mask, wq, wk, wv, wo)
    acc = np.zeros((S, D), dtype=np.float64)
    for r in res.results:
        acc += r["out"].astype(np.float64)
    return acc.astype(np.float32).reshape(B, S, D)
